# revision 1
# baseline (speedup 1.0000x reference)
"""Trainium2 Bass kernel for the sparse-attention CompiledTransformerLayer.

Math (derived from the reference):
  c0 = rowsum(mask0); attended = (mask0 @ x[:,:,0:16]) * r/(1-r), r = 1/(1+c0)
  out ch16:32 = attended @ W_o0.T
  out ch32    = c1 * W_o1[0,0], c1 = rowsum(mask1)
  out ch48:64 = a + b; 64:80 = a*b; 80:96 = (a > b), a = x ch0:16, b = ch16:32
  all other channels pass through from x.

Sharding: 8 cores = 4 batches x 2 query-halves (1024 queries each).

Tricks:
  - bool masks are DMA-transposed as uint16 byte-pairs (HWDGE xbar, 2-byte dtype),
    then fed to the PE matmul directly as float8e4: byte 0x01 is the fp8
    denormal 2^-9, so results are exactly scaled by 2^-9 (weights pre-scaled
    by 512 to compensate).
  - value weights are (x[:,:,0:16] @ W_o0.T) split hi+lo in bf16 for ~f32
    matmul precision; an extra ones*512 column yields c0 in the same psum.
  - rowsum(mask1) via an all-ones fp8 stationary matmul (exact).
"""
import sys
sys.path.insert(0, "/opt/trn_rl_repo")
import numpy as np
import ml_dtypes

import concourse.bass as bass
import concourse.mybir as mybir
from concourse import tile
from concourse.bass_utils import run_bass_kernel_spmd
from concourse.vector_clock import ScopedClock, VectorClock
from concourse.tile import add_dep_helper

B, S, D = 4, 2048, 128
QH = S // 2              # queries per core
NQ = 8                   # j2 blocks of 128 (each covers 256 keys)
DT = mybir.dt
AL = mybir.AluOpType

# walrus codegen rejects instructions with many sem waits; the Tile tail
# drain accumulates one wait per touched proc. Emit one single-wait drain
# per proc instead.
def _patched_dab(self, tick_clock, wait_clock):
    ticks = list(tick_clock.global_clock)
    for i, t in enumerate(ticks):
        if t <= 0:
            continue
        part = [t if j == i else 0 for j, t in enumerate(ticks)]
        d = self.nc.sync.drain()
        wait_clock.add_sem_waits(d.ins, ScopedClock({None: VectorClock(part)}))
    self.nc.sync.drain()
    self.nc.all_engine_barrier()
    popped = self.nc._tile_sem_poison_stack.pop()
    assert popped is self._sem_poison
    self.nc.clear_and_free_semaphores(list(self.sems.allocated().values()))
    self.nc.all_engine_barrier()
tile.TileContext._drain_and_barrier = _patched_dab


def _build_program():
    nc = bass.Bass()
    m0_d = nc.declare_dram_parameter("m0", [QH, S // 2], DT.uint16, isOutput=False)
    m1_d = nc.declare_dram_parameter("m1", [QH, S // 2], DT.uint16, isOutput=False)
    x_d = nc.declare_dram_parameter("xq", [QH, D], DT.float32, isOutput=False)
    whi_d = nc.declare_dram_parameter("whi", [128, NQ, 2, 17], DT.bfloat16, isOutput=False)
    wlo_d = nc.declare_dram_parameter("wlo", [128, NQ, 2, 17], DT.bfloat16, isOutput=False)
    wo1_d = nc.declare_dram_parameter("wo1", [128, 1], DT.float32, isOutput=False)
    out_d = nc.declare_dram_parameter("out", [QH, D], DT.float32, isOutput=True)

    x_view = None  # set below
    with tile.TileContext(nc) as tc, \
         tc.tile_pool(name="const", bufs=1) as cpool, \
         tc.tile_pool(name="masks", bufs=8) as mpool, \
         tc.tile_pool(name="work", bufs=2) as wpool, \
         tc.tile_pool(name="ps", bufs=1, space="PSUM") as ps:

        x_view = x_d[:].rearrange("(t p) c -> p t c", p=128)    # [128, 8, 128]
        o_view = out_d[:].rearrange("(t p) c -> p t c", p=128)

        # x loads first: zero-wait DMAs, and they precede every transpose so
        # the xbar-mode serialization never lands on them
        ots = []
        xdmas = []
        for h in range(2):
            ot = wpool.tile([128, 4, D], DT.float32, tag=f"ot{h}", name=f"ot{h}")
            xdmas.append(nc.sync.dma_start(ot[:], x_view[:, 4 * h:4 * (h + 1), :]))
            ots.append(ot)

        whi = cpool.tile([128, NQ, 2, 17], DT.bfloat16)
        wlo = cpool.tile([128, NQ, 2, 17], DT.bfloat16)
        nc.sync.dma_start(whi[:], whi_d[:])
        nc.sync.dma_start(wlo[:], wlo_d[:])
        wo1_raw = cpool.tile([128, 1], DT.float32)
        nc.sync.dma_start(wo1_raw[:], wo1_d[:])
        wo1 = cpool.tile([128, 1], DT.float32)
        nc.vector.tensor_copy(wo1[:], wo1_raw[:])   # absorb DMA wait off TT path
        ones8 = cpool.tile([128, 32], DT.float8e4)
        nc.vector.memset(ones8[:], 1.0)

        # psum accumulation groups per query-half
        S_ps = [ps.tile([32, 512], DT.float32, tag=f"S{h}", name=f"S{h}") for h in range(2)]
        C_ps = [ps.tile([32, 512], DT.float32, tag=f"C{h}", name=f"C{h}") for h in range(2)]

        # ---- matmul phase: stream mask tiles (all resident), h outer so the
        # h=0 post phase overlaps the h=1 matmuls ----
        m0rs, m1rs = [], []
        for q in range(NQ):
            m0t = mpool.tile([128, QH], DT.uint16, tag="m0")
            m0dma = nc.sync.dma_start(m0t[:], m0_d[:, 128 * q:128 * (q + 1)], transpose=True)
            m1t = mpool.tile([128, QH], DT.uint16, tag="m1")
            m1dma = nc.sync.dma_start(m1t[:], m1_d[:, 128 * q:128 * (q + 1)], transpose=True)
            m0rs.append(m0t[:].bitcast(DT.float8e4).rearrange("p (i two) -> p i two", two=2))
            m1rs.append(m1t[:].bitcast(DT.float8e4).rearrange("p (i two) -> p i two", two=2))
        for h in range(2):
            for q in range(NQ):
                for par in range(2):
                    rhs0 = m0rs[q][:, 512 * h:512 * (h + 1), par]
                    first = (q == 0 and par == 0)
                    last = (q == NQ - 1 and par == 1)
                    nc.tensor.matmul(S_ps[h][0:17, :], whi[:, q, par, :], rhs0,
                                     start=first, stop=False)
                    nc.tensor.matmul(S_ps[h][0:17, :], wlo[:, q, par, :], rhs0,
                                     start=False, stop=last)
                    rhs1 = m1rs[q][:, 512 * h:512 * (h + 1), par]
                    last_mm = nc.tensor.matmul(C_ps[h][:], ones8[:], rhs1,
                                               start=first, stop=last)

        # x loads on Pool/SWDGE after all transpose DMAs (xbar-mode safety,
        # and they double as the xbar fence for the out-stores); only needed
        # in the post phase, so the delay hides under the matmul tail.

        # chain of tiny Pool DMAs, each absorbing exactly one sem for the
        # 1-wait-limited Pool out-stores: xbar serialization, then the two
        # x-load lanes
        xfence = cpool.tile([1, 4], DT.float32, name="xfence")
        f1 = nc.gpsimd.dma_start(xfence[0:1, 0:1], wo1_d[0:1, :])
        add_dep_helper(f1.ins, m1dma.ins, reason="xbar fence after last m1 transpose")
        f1b = nc.gpsimd.dma_start(xfence[0:1, 3:4], wo1_d[0:1, :])
        add_dep_helper(f1b.ins, m0dma.ins, reason="xbar fence after last m0 transpose")
        add_dep_helper(f1b.ins, f1.ins, sync=False, reason="pool order")
        f2 = nc.gpsimd.dma_start(xfence[0:1, 1:2], wo1_d[0:1, :])
        add_dep_helper(f2.ins, xdmas[0].ins, reason="absorb x-load h0 lane")
        add_dep_helper(f2.ins, f1.ins, sync=False, reason="pool order")
        f3 = nc.gpsimd.dma_start(xfence[0:1, 2:3], wo1_d[0:1, :])
        add_dep_helper(f3.ins, xdmas[1].ins, reason="absorb x-load h1 lane")
        add_dep_helper(f3.ins, f2.ins, sync=False, reason="pool order")

        # ---- post phase per query-half ----
        for h in range(2):
            Ssb = wpool.tile([32, 512], DT.float32, tag="Ssb")
            nc.scalar.copy(Ssb[:], S_ps[h][:])
            Csb = wpool.tile([32, 512], DT.float32, tag="Csb")
            nc.scalar.copy(Csb[:], C_ps[h][:])

            TS = wpool.tile([32, 512], DT.float32, tag="TS")
            nc.vector.transpose(TS[:], Ssb[:])      # 16 in-place 32x32 blocks
            TC = wpool.tile([32, 512], DT.float32, tag="TC")
            nc.vector.transpose(TC[:], Csb[:])

            # att[128p, t, d] = S[d, 128t+p]; block (4t+m) of TS holds rows 32m..32m+32
            att = wpool.tile([128, 4, 32], DT.float32, tag="att")
            TSv = TS[:].rearrange("p (k d) -> p k d", d=32)     # [32, 16, 32]
            TCv = TC[:].rearrange("p (k d) -> p k d", d=32)
            for m in range(4):
                nc.vector.tensor_copy(att[32 * m:32 * m + 32, :, :], TSv[:, m::4, :])

            # scale chain on [128, 4]: c0 = att[:, :, 16]; w = r/(1-r), r=1/(1+c0)
            denom = wpool.tile([128, 4], DT.float32, tag="denom")
            nc.vector.tensor_scalar_add(denom[:], att[:, :, 16], 1.0)
            r_t = wpool.tile([128, 4], DT.float32, tag="r_t")
            nc.vector.reciprocal(r_t[:], denom[:])
            omr = wpool.tile([128, 4], DT.float32, tag="omr")
            nc.vector.tensor_scalar(omr[:], r_t[:], -1.0, 1.0, AL.mult, AL.add)
            nc.vector.tensor_scalar_max(omr[:], omr[:], 1e-9)
            romr = wpool.tile([128, 4], DT.float32, tag="romr")
            nc.vector.reciprocal(romr[:], omr[:])
            wcol = wpool.tile([128, 4], DT.float32, tag="wcol")
            nc.vector.tensor_tensor(wcol[:], r_t[:], romr[:], AL.mult)

            # output staging: x rows stream straight into the out tile
            ot = ots[h]
            lab = wpool.tile([1, 1], DT.float32, tag="lab", name=f"lab{h}")
            abs_cp = nc.vector.tensor_copy(lab[:], ot[0:1, 0, 0:1])

            atts = wpool.tile([128, 4, 16], DT.float32, tag="atts")
            for t in range(4):
                nc.vector.scalar_tensor_tensor(
                    atts[:, t, :], att[:, t, 0:16], wcol[:, t:t + 1],
                    att[:, t, 0:16], AL.mult, AL.bypass)

            # ch16:32 = attended
            cp1632 = nc.vector.tensor_copy(ot[:, :, 16:32], atts[:])
            add_dep_helper(cp1632.ins, abs_cp.ins, sync=False, reason="after lane absorb")
            # ch32 = c1 * W_o1 (gather TC blocks to full partitions first)
            c1col = wpool.tile([128, 4], DT.float32, tag="c1col")
            for m in range(4):
                nc.vector.tensor_copy(c1col[32 * m:32 * m + 32, :], TCv[:, m::4, 0])
            cstt = nc.vector.scalar_tensor_tensor(
                ot[:, :, 32:33].rearrange("p t one -> p (t one)"),
                c1col[:], wo1[:], c1col[:], AL.mult, AL.bypass)
            add_dep_helper(cstt.ins, abs_cp.ins, sync=False, reason="after lane absorb")
            # MLP: a = ch0:16, b = atts
            for alu, lo in ((AL.add, 48), (AL.mult, 64), (AL.is_lt, 80)):
                mlp = nc.vector.tensor_tensor(ot[:, :, lo:lo + 16], atts[:],
                                              ot[:, :, 0:16], alu)
                add_dep_helper(mlp.ins, abs_cp.ins, sync=False, reason="after lane absorb")

            nc.gpsimd.dma_start(o_view[:, 4 * h:4 * (h + 1), :], ot[:])

    return nc


_cached = {}


def _prepare_in_maps(x, mask0, mask1, W_o0, W_o1):
    x = np.asarray(x, dtype=np.float32)
    m0u8 = np.asarray(mask0).astype(np.uint8, copy=False)
    m1u8 = np.asarray(mask1).astype(np.uint8, copy=False)
    W_o0 = np.asarray(W_o0, dtype=np.float32)
    W_o1 = np.asarray(W_o1, dtype=np.float32)

    # u = values through the head-0 output projection; hi/lo split, x512
    in_maps = []
    for c in range(8):
        b, h = divmod(c, 2)
        u = x[b, :, 0:16] @ W_o0.T                      # (S, 16) f32
        u_hi = u.astype(ml_dtypes.bfloat16)
        u_lo = (u - u_hi.astype(np.float32)).astype(ml_dtypes.bfloat16)
        whi = np.zeros((128, NQ, 2, 17), dtype=ml_dtypes.bfloat16)
        wlo = np.zeros((128, NQ, 2, 17), dtype=ml_dtypes.bfloat16)
        for q in range(NQ):
            blk_hi = u_hi[256 * q:256 * (q + 1)]        # (256, 16)
            blk_lo = u_lo[256 * q:256 * (q + 1)]
            for par in range(2):
                whi[:, q, par, :16] = (blk_hi[par::2].astype(np.float32) * 512.0
                                       ).astype(ml_dtypes.bfloat16)
                wlo[:, q, par, :16] = (blk_lo[par::2].astype(np.float32) * 512.0
                                       ).astype(ml_dtypes.bfloat16)
            whi[:, q, :, 16] = 512.0                    # ones column -> c0 exactly
        sl = slice(QH * h, QH * (h + 1))
        in_maps.append({
            "m0": np.ascontiguousarray(m0u8[b, sl, :]).view(np.uint16),
            "m1": np.ascontiguousarray(m1u8[b, sl, :]).view(np.uint16),
            "xq": np.ascontiguousarray(x[b, sl, :]),
            "whi": whi,
            "wlo": wlo,
            "wo1": np.full((128, 1), 512.0 * float(W_o1[0, 0]), np.float32),
        })
    return in_maps


def kernel(x, mask0, mask1, W_o0, W_o1):
    if "nc" not in _cached:
        _cached["nc"] = _build_program()
    nc = _cached["nc"]
    in_maps = _prepare_in_maps(x, mask0, mask1, W_o0, W_o1)
    res = run_bass_kernel_spmd(nc, in_maps, list(range(8)))
    _cached["last_results"] = res
    out = np.empty((B, S, D), np.float32)
    for c in range(8):
        b, h = divmod(c, 2)
        out[b, QH * h:QH * (h + 1), :] = res.results[c]["out"]
    return out



# revision 27
# speedup vs baseline: 1.4991x; 1.4991x over previous
"""Trainium2 Bass kernel for the sparse-attention CompiledTransformerLayer.

Math (derived from the reference):
  c0 = rowsum(mask0); attended = (mask0 @ x[:,:,0:16]) * r/(1-r), r = 1/(1+c0)
  out ch16:32 = attended @ W_o0.T
  out ch32    = c1 * W_o1[0,0], c1 = rowsum(mask1)
  out ch48:64 = a + b; 64:80 = a*b; 80:96 = (a > b), a = x ch0:16, b = ch16:32
  all other channels pass through from x (merged on the host).

Sharding: 8 cores = 4 batches x 2 query-halves (1024 queries each).

Key tricks:
  - nibble packing: the host packs BOTH masks for two adjacent keys into one
    byte  p = m0[2j] + 2*m0[2j+1] + 4*(m1[2j]+m1[2j+1]).  fp8e4 decodes bytes
    0..15 exactly as k*2^-9, so one transposed load (1MB/core instead of 4MB)
    carries all mask information.
  - the packed bytes are DMA-transposed as uint16 (HWDGE xbar), then two DVE
    shift/and ops recover the mask0 even/odd key bit-planes (values 0x00/0x01
    = fp8 0 / 2^-9).
  - matmuls are flipped vs the usual attention layout: a 128x128 mask^T block
    is the *stationary* operand and the 34-wide value vector
    [u_hi | u_lo | ones | gones] is the *moving* operand, so each matmul only
    streams 34 columns.  PSUM ends up as [query, channel] - no transposes.
  - u = x[:,:,0:16] @ W_o0.T is precomputed on the host, split hi+lo in bf16
    (scaled by 512 to cancel the fp8 2^-9), giving ~f32 matmul precision.
  - c1 falls out linearly: a raw-packed-byte matmul against ones gives
    T = g + 4*c1 where g = c0_even + 2*c0_odd comes from the 34th moving
    column (weight 1 on the even plane, 2 on the odd plane).
"""
import sys
sys.path.insert(0, "/opt/trn_rl_repo")
import numpy as np
import ml_dtypes

import concourse.bass as bass
import concourse.mybir as mybir
from concourse import tile
from concourse.bass_utils import run_bass_kernel_spmd
from concourse.vector_clock import ScopedClock, VectorClock
from concourse.tile import add_dep_helper

B, S, D = 4, 2048, 128
QH = S // 2              # queries per core
NCH = 4                  # transpose chunks (128 u16 cols = 512 keys each)
NIB = QH // 128          # query blocks per core (8)
DT = mybir.dt
AL = mybir.AluOpType

# walrus codegen rejects instructions with many sem waits; the Tile tail
# drain accumulates one wait per touched proc. Emit one single-wait drain
# per proc instead.
def _patched_dab(self, tick_clock, wait_clock):
    ticks = list(tick_clock.global_clock)
    for i, t in enumerate(ticks):
        if t <= 0:
            continue
        part = [t if j == i else 0 for j, t in enumerate(ticks)]
        d = self.nc.sync.drain()
        wait_clock.add_sem_waits(d.ins, ScopedClock({None: VectorClock(part)}))
    self.nc.sync.drain()
    self.nc.all_engine_barrier()
    popped = self.nc._tile_sem_poison_stack.pop()
    assert popped is self._sem_poison
    self.nc.clear_and_free_semaphores(list(self.sems.allocated().values()))
    self.nc.all_engine_barrier()
tile.TileContext._drain_and_barrier = _patched_dab


def _build_program():
    nc = bass.Bass()
    mp_d = nc.declare_dram_parameter("mp", [QH, S // 4], DT.uint16, isOutput=False)
    xq_d = nc.declare_dram_parameter("xq", [128, NIB, 16], DT.float32, isOutput=False)
    w3_d = nc.declare_dram_parameter("w3", [128, NCH, 2, 2, 2, 17], DT.bfloat16, isOutput=False)
    cone_d = nc.declare_dram_parameter("cone", [128, 3], DT.bfloat16, isOutput=False)
    cwo1_d = nc.declare_dram_parameter("cwo1", [128, 1], DT.float32, isOutput=False)
    out_d = nc.declare_dram_parameter("out", [128, NIB, 65], DT.float32, isOutput=True)

    with tile.TileContext(nc) as tc, \
         tc.tile_pool(name="const", bufs=1) as cpool, \
         tc.tile_pool(name="masks", bufs=1) as mpool, \
         tc.tile_pool(name="work", bufs=1) as wpool, \
         tc.tile_pool(name="ps", bufs=1, space="PSUM") as ps:

        # small loads via Pool SWDGE so HWDGE stays free for the transposes
        w3 = cpool.tile([128, NCH, 2, 2, 2, 17], DT.bfloat16)
        nc.gpsimd.dma_start(w3[:], w3_d[:])
        cone = cpool.tile([128, 3], DT.bfloat16)
        nc.gpsimd.dma_start(cone[:], cone_d[:])
        cwo1_raw = cpool.tile([128, 1], DT.float32)
        nc.gpsimd.dma_start(cwo1_raw[:], cwo1_d[:])
        cwo1 = cpool.tile([128, 1], DT.float32)
        nc.vector.tensor_copy(cwo1[:], cwo1_raw[:])   # absorb DMA wait off STT path
        xq = cpool.tile([128, NIB, 16], DT.float32)
        nc.gpsimd.dma_start(xq[:], xq_d[:])
        lab = cpool.tile([1, 1], DT.float32)
        nc.vector.tensor_copy(lab[:], xq[0:1, 0, 0:1])   # absorb xq DMA wait on DVE
        labw = cpool.tile([1, 2], DT.bfloat16)
        nc.vector.tensor_copy(labw[0:1, 0:1], w3[0:1, 0, 0, 0, 0, 0:1])  # absorb w3
        nc.vector.tensor_copy(labw[0:1, 1:2], cone[0:1, 0:1])            # absorb cone

        # start=True resets psum at larger-than-region granularity, so zero the
        # whole accumulator with one dummy all-zeros matmul and use start=False
        # (pure accumulate) for every real matmul.  One tile holds S (cols
        # 0:16 att, 16 c0) and C (col 17, T-g) so all writers are PE.
        P_ps = ps.tile([128, NIB, 18], DT.float32, tag="P", name="P")
        zmv = cpool.tile([128, 160], DT.bfloat16)
        nc.vector.memset(zmv[:], 0.0)
        nc.tensor.matmul(P_ps[:].rearrange("p a b -> p (a b)"), zmv[:, 0:128],
                         zmv[:, 0:NIB * 18], start=True, stop=False,
                         skip_group_check=True)

        last_tdma = None
        for t in range(NCH):
            mt = mpool.tile([128, QH], DT.uint16, tag=f"mt{t}", name=f"mt{t}")
            last_tdma = nc.sync.dma_start(mt[:], mp_d[:, 128 * t:128 * (t + 1)],
                                          transpose=True)
            # bit-plane extracts: even keys = bit0, odd keys = bit1 (per byte)
            ev = mpool.tile([128, QH], DT.uint16, tag=f"ev{t}", name=f"ev{t}")
            nc.vector.tensor_scalar(ev[:], mt[:], 0x0101, 0, AL.bitwise_and,
                                    AL.bitwise_or)
            od = mpool.tile([128, QH], DT.uint16, tag=f"od{t}", name=f"od{t}")
            odx = nc.vector.tensor_scalar(od[:], mt[:], 1, 0x0101,
                                          AL.logical_shift_right, AL.bitwise_and)

            mr = mt[:].bitcast(DT.float8e4).rearrange("p (i two) -> p i two", two=2)
            evr = ev[:].bitcast(DT.float8e4).rearrange("p (i two) -> p i two", two=2)
            odr = od[:].bitcast(DT.float8e4).rearrange("p (i two) -> p i two", two=2)

            last = (t == NCH - 1)
            for par in range(2):
                for e, pl in ((0, evr), (1, odr)):
                    stops = (last and par == 1 and e == 1)
                    for hl in range(2):
                        mv = w3[:, t, par, e, hl, :]
                        for ib in range(NIB):
                            nc.tensor.matmul(
                                P_ps[:, ib, 0:17],
                                pl[:, 128 * ib:128 * (ib + 1), par], mv,
                                start=False,
                                stop=(stops and hl == 1 and ib == NIB - 1),
                                skip_group_check=True)
                    # -g accumulation: C -= (1+e) * c0-plane contribution
                    for ib in range(NIB):
                        nc.tensor.matmul(
                            P_ps[:, ib, 17:18],
                            pl[:, 128 * ib:128 * (ib + 1), par],
                            cone[:, 1 + e:2 + e],
                            start=False, stop=False,
                            skip_group_check=True)
                # raw packed bytes vs ones: C += g + 4*c1
                for ib in range(NIB):
                    cmm = nc.tensor.matmul(
                        P_ps[:, ib, 17:18], mr[:, 128 * ib:128 * (ib + 1), par],
                        cone[:, 0:1],
                        start=False,
                        stop=(last and par == 1 and ib == NIB - 1),
                        skip_group_check=True)
                    # route deps through the chunk's DVE extract so the wait
                    # set collapses to a single DVE sem (covers mt + cone)
                    add_dep_helper(cmm.ins, odx.ins, reason="chunk ready")

        # ---- tail: scale chain + MLP, all in [128 queries, NIB, ch] layout --
        denom = wpool.tile([128, NIB], DT.float32, tag="denom")
        nc.vector.tensor_scalar_add(denom[:], P_ps[:, :, 16], 1.0)
        r_t = wpool.tile([128, NIB], DT.float32, tag="r_t")
        nc.vector.reciprocal(r_t[:], denom[:])
        omr = wpool.tile([128, NIB], DT.float32, tag="omr")
        nc.vector.tensor_scalar(omr[:], r_t[:], -1.0, 1.0, AL.mult, AL.add)
        nc.vector.tensor_scalar_max(omr[:], omr[:], 1e-9)
        romr = wpool.tile([128, NIB], DT.float32, tag="romr")
        nc.vector.reciprocal(romr[:], omr[:])
        wcol = wpool.tile([128, NIB], DT.float32, tag="wcol")
        nc.vector.tensor_tensor(wcol[:], r_t[:], romr[:], AL.mult)

        ot = wpool.tile([128, NIB, 65], DT.float32, tag="ot")
        wb = wcol[:].unsqueeze(2).broadcast_to([128, NIB, 16])
        nc.vector.tensor_tensor(ot[:, :, 0:16], P_ps[:, :, 0:16], wb, AL.mult)
        # count: c1*W_o1 = (T - g) * (W_o1/4), C psum already holds T - g
        nc.vector.scalar_tensor_tensor(ot[:, :, 16], P_ps[:, :, 17], cwo1[:], wcol[:],
                                       AL.mult, AL.bypass)
        # MLP: a = x ch0:16, b = attended
        nc.vector.tensor_tensor(ot[:, :, 17:33], ot[:, :, 0:16], xq[:], AL.add)
        nc.vector.tensor_tensor(ot[:, :, 33:49], ot[:, :, 0:16], xq[:], AL.mult)
        nc.vector.tensor_tensor(ot[:, :, 49:65], ot[:, :, 0:16], xq[:], AL.is_lt)

        # tiny pool fence absorbs the transpose-DMA sem so the out-store
        # carries a single (DVE) wait — walrus limits DMA sem waits to one
        fence = cpool.tile([1, 1], DT.float32)
        fdma = nc.gpsimd.dma_start(fence[0:1, 0:1], cwo1_d[0:1, :])
        add_dep_helper(fdma.ins, last_tdma.ins, reason="absorb xbar/hwdge sem")
        sdma = nc.gpsimd.dma_start(out_d[:], ot[:])
        add_dep_helper(sdma.ins, fdma.ins, sync=False, reason="pool order")

    return nc


_cached = {}


def _prepare_in_maps(x, mask0, mask1, W_o0, W_o1):
    x = np.asarray(x, dtype=np.float32)
    m0 = np.asarray(mask0).view(np.uint8)
    m1 = np.asarray(mask1).view(np.uint8)
    W_o0 = np.asarray(W_o0, dtype=np.float32)
    W_o1 = np.asarray(W_o1, dtype=np.float32)

    # nibble pack: byte jj = m0[2jj] + 2*m0[2jj+1] + 4*(m1[2jj]+m1[2jj+1])
    packed = (m0[..., 0::2] + (m0[..., 1::2] << 1)
              + ((m1[..., 0::2] + m1[..., 1::2]) << 2))        # (B, S, S//2) u8

    # u = values through the head-0 output projection; hi/lo split, x512
    u = x[:, :, 0:16] @ W_o0.T                                 # (B, S, 16) f32
    u_hi = u.astype(ml_dtypes.bfloat16).astype(np.float32)
    u_lo = u - u_hi

    # key index per (partition, chunk, byte-lane, parity)
    p_i = np.arange(128)[:, None, None, None]
    t_i = np.arange(NCH)[None, :, None, None]
    par_i = np.arange(2)[None, None, :, None]
    e_i = np.arange(2)[None, None, None, :]
    J = 512 * t_i + 4 * p_i + 2 * par_i + e_i                  # [128,4,2,2]

    w3s = []
    for b in range(B):
        w3 = np.zeros((128, NCH, 2, 2, 2, 17), dtype=ml_dtypes.bfloat16)
        w3[..., 0, 0:16] = (512.0 * u_hi[b][J]).astype(ml_dtypes.bfloat16)
        w3[..., 1, 0:16] = (512.0 * u_lo[b][J]).astype(ml_dtypes.bfloat16)
        w3[..., 0, 16] = 512.0                                 # ones -> c0
        w3s.append(w3)

    # cone cols: +512 (raw bytes -> g + 4*c1), -512 / -1024 (minus c0e / 2*c0o)
    cone = np.array([[512.0, -512.0, -1024.0]] * 128, dtype=ml_dtypes.bfloat16)
    cwo1 = np.full((128, 1), float(W_o1[0, 0]) / 4.0, dtype=np.float32)

    in_maps = []
    for c in range(8):
        b, h = divmod(c, 2)
        sl = slice(QH * h, QH * (h + 1))
        xq = np.ascontiguousarray(
            x[b, sl, 0:16].reshape(NIB, 128, 16).transpose(1, 0, 2))
        in_maps.append({
            "mp": np.ascontiguousarray(packed[b, sl, :]).view(np.uint16),
            "xq": xq,
            "w3": w3s[b],
            "cone": cone,
            "cwo1": cwo1,
        })
    return in_maps


def kernel(x, mask0, mask1, W_o0, W_o1):
    if "nc" not in _cached:
        _cached["nc"] = _build_program()
    nc = _cached["nc"]
    in_maps = _prepare_in_maps(x, mask0, mask1, W_o0, W_o1)
    res = run_bass_kernel_spmd(nc, in_maps, list(range(8)))
    _cached["last_results"] = res
    out = np.array(np.asarray(x, dtype=np.float32), copy=True)
    for c in range(8):
        b, h = divmod(c, 2)
        sl = slice(QH * h, QH * (h + 1))
        rows = res.results[c]["out"].transpose(1, 0, 2).reshape(QH, 65)
        out[b, sl, 16:32] = rows[:, 0:16]
        out[b, sl, 32] = rows[:, 16]
        out[b, sl, 48:96] = rows[:, 17:65]
    return out


# revision 29
# speedup vs baseline: 2.8722x; 1.9159x over previous
"""Trainium2 Bass kernel for the sparse-attention CompiledTransformerLayer.

Math (derived from the reference):
  c0 = rowsum(mask0); attended = (mask0 @ x[:,:,0:16]) * r/(1-r), r = 1/(1+c0)
  out ch16:32 = attended @ W_o0.T
  out ch32    = c1 * W_o1[0,0], c1 = rowsum(mask1)
  out ch48:64 = a + b; 64:80 = a*b; 80:96 = (a > b), a = x ch0:16, b = ch16:32
  all other channels pass through from x (merged on the host).

Sharding: 8 cores = 4 batches x 2 query-halves (1024 queries each).

Key tricks:
  - nibble packing: the host packs BOTH masks for two adjacent keys into one
    byte  p = m0[2j] + 2*m0[2j+1] + 4*(m1[2j]+m1[2j+1]).  fp8e4 decodes bytes
    0..15 exactly as k*2^-9, so one transposed load (1MB/core instead of 4MB)
    carries all mask information.
  - the packed bytes are DMA-transposed as uint16 (HWDGE xbar), then two DVE
    shift/and ops recover the mask0 even/odd key bit-planes (values 0x00/0x01
    = fp8 0 / 2^-9).
  - matmuls are flipped vs the usual attention layout: a 128x128 mask^T block
    is the *stationary* operand and the 34-wide value vector
    [u_hi | u_lo | ones | gones] is the *moving* operand, so each matmul only
    streams 34 columns.  PSUM ends up as [query, channel] - no transposes.
  - u = x[:,:,0:16] @ W_o0.T is precomputed on the host, split hi+lo in bf16
    (scaled by 512 to cancel the fp8 2^-9), giving ~f32 matmul precision.
  - c1 falls out linearly: a raw-packed-byte matmul against ones gives
    T = g + 4*c1 where g = c0_even + 2*c0_odd comes from the 34th moving
    column (weight 1 on the even plane, 2 on the odd plane).
"""
import sys
sys.path.insert(0, "/opt/trn_rl_repo")
import numpy as np
import ml_dtypes

import concourse.bass as bass
import concourse.mybir as mybir
from concourse import tile
from concourse.bass_utils import run_bass_kernel_spmd
from concourse.vector_clock import ScopedClock, VectorClock
from concourse.tile import add_dep_helper

B, S, D = 4, 2048, 128
QH = S // 2              # queries per core
NCH = 4                  # transpose chunks (128 u16 cols = 512 keys each)
NIB = QH // 128          # query blocks per core (8)
DT = mybir.dt
AL = mybir.AluOpType

# walrus codegen rejects instructions with many sem waits; the Tile tail
# drain accumulates one wait per touched proc. Emit one single-wait drain
# per proc instead.
def _patched_dab(self, tick_clock, wait_clock):
    ticks = list(tick_clock.global_clock)
    for i, t in enumerate(ticks):
        if t <= 0:
            continue
        part = [t if j == i else 0 for j, t in enumerate(ticks)]
        d = self.nc.sync.drain()
        wait_clock.add_sem_waits(d.ins, ScopedClock({None: VectorClock(part)}))
    self.nc.sync.drain()
    self.nc.all_engine_barrier()
    popped = self.nc._tile_sem_poison_stack.pop()
    assert popped is self._sem_poison
    self.nc.clear_and_free_semaphores(list(self.sems.allocated().values()))
    self.nc.all_engine_barrier()
tile.TileContext._drain_and_barrier = _patched_dab


def _build_program():
    nc = bass.Bass()
    mp_d = nc.declare_dram_parameter("mp", [QH, S // 4], DT.uint16, isOutput=False)
    blob_d = nc.declare_dram_parameter("blob", [832, 128], DT.uint16, isOutput=False)
    out_d = nc.declare_dram_parameter("out", [128, NIB, 65], DT.float32, isOutput=True)

    with tile.TileContext(nc) as tc, \
         tc.tile_pool(name="const", bufs=1) as cpool, \
         tc.tile_pool(name="masks", bufs=1) as mpool, \
         tc.tile_pool(name="work", bufs=1) as wpool, \
         tc.tile_pool(name="ps", bufs=1, space="PSUM") as ps:

        # every input load goes through the xbar (transpose DMA): mixing
        # regular and transpose DMAs forces a serializing mode-switch fence
        # between each pair, so the small inputs ride one pre-transposed blob.
        blob_t = mpool.tile([128, 832], DT.uint16, tag="blob", name="blob")
        nc.sync.dma_start(blob_t[:], blob_d[:], transpose=True)
        # DVE-launder the blob so every consumer dep collapses onto DVE sems
        blob2 = cpool.tile([128, 806], DT.uint16)
        nc.vector.tensor_copy(blob2[:], blob_t[:, 0:806])
        w3 = blob2[:, 0:544].bitcast(DT.bfloat16).rearrange(
            "p (t a b c d) -> p t a b c d", t=NCH, a=2, b=2, c=2, d=17)
        xq = blob2[:, 544:800].bitcast(DT.float32).rearrange(
            "p (i c) -> p i c", i=NIB, c=16)
        cwo1 = blob2[:, 800:802].bitcast(DT.float32)
        cone = blob2[:, 802:805].bitcast(DT.bfloat16)

        # start=True resets psum at larger-than-region granularity, so zero the
        # whole accumulator with one dummy all-zeros matmul and use start=False
        # (pure accumulate) for every real matmul.  One tile holds S (cols
        # 0:16 att, 16 c0) and C (col 17, T-g) so all writers are PE.
        P_ps = ps.tile([128, NIB, 18], DT.float32, tag="P", name="P")
        zmv = cpool.tile([128, 160], DT.bfloat16)
        nc.vector.memset(zmv[:], 0.0)
        nc.tensor.matmul(P_ps[:].rearrange("p a b -> p (a b)"), zmv[:, 0:128],
                         zmv[:, 0:NIB * 18], start=True, stop=False,
                         skip_group_check=True)

        last_tdma = None
        for t in range(NCH):
            mt = mpool.tile([128, QH], DT.uint16, tag=f"mt{t}", name=f"mt{t}")
            last_tdma = nc.sync.dma_start(mt[:], mp_d[:, 128 * t:128 * (t + 1)],
                                          transpose=True)
            # bit-plane extracts: even keys = bit0, odd keys = bit1 (per byte)
            ev = mpool.tile([128, QH], DT.uint16, tag=f"ev{t}", name=f"ev{t}")
            nc.vector.tensor_scalar(ev[:], mt[:], 0x0101, 0, AL.bitwise_and,
                                    AL.bitwise_or)
            od = mpool.tile([128, QH], DT.uint16, tag=f"od{t}", name=f"od{t}")
            odx = nc.vector.tensor_scalar(od[:], mt[:], 1, 0x0101,
                                          AL.logical_shift_right, AL.bitwise_and)

            mr = mt[:].bitcast(DT.float8e4).rearrange("p (i two) -> p i two", two=2)
            evr = ev[:].bitcast(DT.float8e4).rearrange("p (i two) -> p i two", two=2)
            odr = od[:].bitcast(DT.float8e4).rearrange("p (i two) -> p i two", two=2)

            last = (t == NCH - 1)
            for par in range(2):
                for e, pl in ((0, evr), (1, odr)):
                    stops = (last and par == 1 and e == 1)
                    for hl in range(2):
                        mv = w3[:, t, par, e, hl, :]
                        for ib in range(NIB):
                            nc.tensor.matmul(
                                P_ps[:, ib, 0:17],
                                pl[:, 128 * ib:128 * (ib + 1), par], mv,
                                start=False,
                                stop=(stops and hl == 1 and ib == NIB - 1),
                                skip_group_check=True)
                    # -g accumulation: C -= (1+e) * c0-plane contribution
                    for ib in range(NIB):
                        nc.tensor.matmul(
                            P_ps[:, ib, 17:18],
                            pl[:, 128 * ib:128 * (ib + 1), par],
                            cone[:, 1 + e:2 + e],
                            start=False, stop=False,
                            skip_group_check=True)
                # raw packed bytes vs ones: C += g + 4*c1
                for ib in range(NIB):
                    cmm = nc.tensor.matmul(
                        P_ps[:, ib, 17:18], mr[:, 128 * ib:128 * (ib + 1), par],
                        cone[:, 0:1],
                        start=False,
                        stop=(last and par == 1 and ib == NIB - 1),
                        skip_group_check=True)
                    # route deps through the chunk's DVE extract so the wait
                    # set collapses to a single DVE sem (covers mt + cone)
                    add_dep_helper(cmm.ins, odx.ins, reason="chunk ready")

        # ---- tail: scale chain + MLP, all in [128 queries, NIB, ch] layout --
        denom = wpool.tile([128, NIB], DT.float32, tag="denom")
        nc.vector.tensor_scalar_add(denom[:], P_ps[:, :, 16], 1.0)
        r_t = wpool.tile([128, NIB], DT.float32, tag="r_t")
        nc.vector.reciprocal(r_t[:], denom[:])
        omr = wpool.tile([128, NIB], DT.float32, tag="omr")
        nc.vector.tensor_scalar(omr[:], r_t[:], -1.0, 1.0, AL.mult, AL.add)
        nc.vector.tensor_scalar_max(omr[:], omr[:], 1e-9)
        romr = wpool.tile([128, NIB], DT.float32, tag="romr")
        nc.vector.reciprocal(romr[:], omr[:])
        wcol = wpool.tile([128, NIB], DT.float32, tag="wcol")
        nc.vector.tensor_tensor(wcol[:], r_t[:], romr[:], AL.mult)

        ot = wpool.tile([128, NIB, 65], DT.float32, tag="ot")
        wb = wcol[:].unsqueeze(2).broadcast_to([128, NIB, 16])
        nc.vector.tensor_tensor(ot[:, :, 0:16], P_ps[:, :, 0:16], wb, AL.mult)
        # count: c1*W_o1 = (T - g) * (W_o1/4), C psum already holds T - g
        nc.vector.scalar_tensor_tensor(ot[:, :, 16], P_ps[:, :, 17], cwo1, wcol[:],
                                       AL.mult, AL.bypass)
        # MLP: a = x ch0:16, b = attended
        nc.vector.tensor_tensor(ot[:, :, 17:33], ot[:, :, 0:16], xq, AL.add)
        nc.vector.tensor_tensor(ot[:, :, 33:49], ot[:, :, 0:16], xq, AL.mult)
        nc.vector.tensor_tensor(ot[:, :, 49:65], ot[:, :, 0:16], xq, AL.is_lt)

        # tiny fence absorbs the transpose-DMA (xbar mode-switch) sem so the
        # out-store carries a single (DVE) wait — walrus allows one per DMA
        fence = cpool.tile([1, 128], DT.uint16)
        fdma = nc.sync.dma_start(fence[0:1, :], blob_d[0:1, :])
        add_dep_helper(fdma.ins, last_tdma.ins, reason="absorb xbar/hwdge sem")
        sdma = nc.sync.dma_start(out_d[:], ot[:])
        add_dep_helper(sdma.ins, fdma.ins, sync=False, reason="queue order")

    return nc


_cached = {}


def _prepare_in_maps(x, mask0, mask1, W_o0, W_o1):
    x = np.asarray(x, dtype=np.float32)
    m0 = np.asarray(mask0).view(np.uint8)
    m1 = np.asarray(mask1).view(np.uint8)
    W_o0 = np.asarray(W_o0, dtype=np.float32)
    W_o1 = np.asarray(W_o1, dtype=np.float32)

    # nibble pack: byte jj = m0[2jj] + 2*m0[2jj+1] + 4*(m1[2jj]+m1[2jj+1])
    packed = (m0[..., 0::2] + (m0[..., 1::2] << 1)
              + ((m1[..., 0::2] + m1[..., 1::2]) << 2))        # (B, S, S//2) u8

    # u = values through the head-0 output projection; hi/lo split, x512
    u = x[:, :, 0:16] @ W_o0.T                                 # (B, S, 16) f32
    u_hi = u.astype(ml_dtypes.bfloat16).astype(np.float32)
    u_lo = u - u_hi

    # key index per (partition, chunk, byte-lane, parity)
    p_i = np.arange(128)[:, None, None, None]
    t_i = np.arange(NCH)[None, :, None, None]
    par_i = np.arange(2)[None, None, :, None]
    e_i = np.arange(2)[None, None, None, :]
    J = 512 * t_i + 4 * p_i + 2 * par_i + e_i                  # [128,4,2,2]

    # cone cols: +512 (raw bytes -> g + 4*c1), -512 / -1024 (minus c0e / 2*c0o)
    cone = np.array([[512.0, -512.0, -1024.0]] * 128, dtype=ml_dtypes.bfloat16)
    cwo1 = np.full((128, 1), float(W_o1[0, 0]) / 4.0, dtype=np.float32)

    blobs = []
    for b in range(B):
        w3 = np.zeros((128, NCH, 2, 2, 2, 17), dtype=ml_dtypes.bfloat16)
        w3[..., 0, 0:16] = (512.0 * u_hi[b][J]).astype(ml_dtypes.bfloat16)
        w3[..., 1, 0:16] = (512.0 * u_lo[b][J]).astype(ml_dtypes.bfloat16)
        w3[..., 0, 16] = 512.0                                 # ones -> c0
        blobs.append(w3)

    in_maps = []
    for c in range(8):
        b, h = divmod(c, 2)
        sl = slice(QH * h, QH * (h + 1))
        xq = np.ascontiguousarray(
            x[b, sl, 0:16].reshape(NIB, 128, 16).transpose(1, 0, 2))
        blob = np.zeros((128, 1664), np.uint8)
        blob[:, 0:1088] = blobs[b].reshape(128, 544).view(np.uint8)
        blob[:, 1088:1600] = xq.reshape(128, 128).view(np.uint8)
        blob[:, 1600:1604] = cwo1.view(np.uint8)
        blob[:, 1604:1610] = cone.view(np.uint8)
        blob_t = np.ascontiguousarray(blob.view(np.uint16).T)  # [832, 128]
        in_maps.append({
            "mp": np.ascontiguousarray(packed[b, sl, :]).view(np.uint16),
            "blob": blob_t,
        })
    return in_maps


def kernel(x, mask0, mask1, W_o0, W_o1):
    if "nc" not in _cached:
        _cached["nc"] = _build_program()
    nc = _cached["nc"]
    in_maps = _prepare_in_maps(x, mask0, mask1, W_o0, W_o1)
    res = run_bass_kernel_spmd(nc, in_maps, list(range(8)))
    _cached["last_results"] = res
    out = np.array(np.asarray(x, dtype=np.float32), copy=True)
    for c in range(8):
        b, h = divmod(c, 2)
        sl = slice(QH * h, QH * (h + 1))
        rows = res.results[c]["out"].transpose(1, 0, 2).reshape(QH, 65)
        out[b, sl, 16:32] = rows[:, 0:16]
        out[b, sl, 32] = rows[:, 16]
        out[b, sl, 48:96] = rows[:, 17:65]
    return out


# revision 36
# speedup vs baseline: 2.9689x; 1.0337x over previous
"""Trainium2 Bass kernel for the sparse-attention CompiledTransformerLayer.

Math (derived from the reference):
  c0 = rowsum(mask0); attended = (mask0 @ x[:,:,0:16]) * r/(1-r), r = 1/(1+c0)
  out ch16:32 = attended @ W_o0.T
  out ch32    = c1 * W_o1[0,0], c1 = rowsum(mask1)
  out ch48:64 = a + b; 64:80 = a*b; 80:96 = (a > b), a = x ch0:16, b = ch16:32
  all other channels pass through from x (merged on the host).

Sharding: 8 cores = 4 batches x 2 query-halves (1024 queries each).

Key tricks:
  - nibble packing: the host packs BOTH masks for two adjacent keys into one
    byte  p = m0[2j] + 2*m0[2j+1] + 4*(m1[2j]+m1[2j+1]).  fp8e4 decodes bytes
    0..15 exactly as k*2^-9, so one transposed load (1MB/core instead of 4MB)
    carries all mask information.
  - the packed bytes are DMA-transposed as uint16 (HWDGE xbar), then two DVE
    shift/and ops recover the mask0 even/odd key bit-planes (values 0x00/0x01
    = fp8 0 / 2^-9).
  - matmuls are flipped vs the usual attention layout: a 128x128 mask^T block
    is the *stationary* operand and the 34-wide value vector
    [u_hi | u_lo | ones | gones] is the *moving* operand, so each matmul only
    streams 34 columns.  PSUM ends up as [query, channel] - no transposes.
  - u = x[:,:,0:16] @ W_o0.T is precomputed on the host, split hi+lo in bf16
    (scaled by 512 to cancel the fp8 2^-9), giving ~f32 matmul precision.
  - c1 falls out linearly: a raw-packed-byte matmul against ones gives
    T = g + 4*c1 where g = c0_even + 2*c0_odd comes from the 34th moving
    column (weight 1 on the even plane, 2 on the odd plane).
"""
import sys
sys.path.insert(0, "/opt/trn_rl_repo")
import numpy as np
import ml_dtypes

import concourse.bass as bass
import concourse.mybir as mybir
from concourse import tile
from concourse.bass_utils import run_bass_kernel_spmd
from concourse.vector_clock import ScopedClock, VectorClock
from concourse.tile import add_dep_helper

B, S, D = 4, 2048, 128
QH = S // 2              # queries per core
NCH = 4                  # transpose chunks (128 u16 cols = 512 keys each)
NIB = QH // 128          # query blocks per core (8)
DT = mybir.dt
AL = mybir.AluOpType

# walrus codegen rejects instructions with many sem waits; the Tile tail
# drain accumulates one wait per touched proc. Emit one single-wait drain
# per proc instead.
def _patched_dab(self, tick_clock, wait_clock):
    ticks = list(tick_clock.global_clock)
    for i, t in enumerate(ticks):
        if t <= 0:
            continue
        part = [t if j == i else 0 for j, t in enumerate(ticks)]
        d = self.nc.sync.drain()
        wait_clock.add_sem_waits(d.ins, ScopedClock({None: VectorClock(part)}))
    self.nc.sync.drain()
    self.nc.all_engine_barrier()
    popped = self.nc._tile_sem_poison_stack.pop()
    assert popped is self._sem_poison
    self.nc.clear_and_free_semaphores(list(self.sems.allocated().values()))
    self.nc.all_engine_barrier()
tile.TileContext._drain_and_barrier = _patched_dab


def _build_program():
    nc = bass.Bass()
    mp_d = nc.declare_dram_parameter("mp", [QH, S // 4], DT.uint16, isOutput=False)
    blob_d = nc.declare_dram_parameter("blob", [832, 128], DT.uint16, isOutput=False)
    outa_d = nc.declare_dram_parameter("outa", [128, NIB, 48], DT.float32, isOutput=True)
    outb_d = nc.declare_dram_parameter("outb", [128, NIB, 17], DT.float32, isOutput=True)

    # (chunk, row0, rows, ib0): last key-chunk split into query halves so the
    # final extract+matmul straggler after the last transpose is half-sized
    PIECES = [(0, 0, QH, 0), (1, 0, QH, 0), (2, 0, QH, 0),
              (3, 0, QH // 2, 0), (3, QH // 2, QH // 2, NIB // 2)]

    with tile.TileContext(nc) as tc, \
         tc.tile_pool(name="const", bufs=1) as cpool, \
         tc.tile_pool(name="masks", bufs=1) as mpool, \
         tc.tile_pool(name="work", bufs=1) as wpool, \
         tc.tile_pool(name="ps", bufs=1, space="PSUM") as ps:

        # every input load goes through the xbar (transpose DMA): mixing
        # regular and transpose DMAs forces a serializing mode-switch fence
        # between each pair, so the small inputs ride one pre-transposed blob.
        blob_t = mpool.tile([128, 832], DT.uint16, tag="blob", name="blob")
        nc.sync.dma_start(blob_t[:], blob_d[:], transpose=True)
        # DVE-launder the blob so every consumer dep collapses onto DVE sems
        blob2 = cpool.tile([128, 806], DT.uint16)
        nc.vector.tensor_copy(blob2[:], blob_t[:, 0:806])
        w3 = blob2[:, 0:544].bitcast(DT.bfloat16).rearrange(
            "p (t a b c d) -> p t a b c d", t=NCH, a=2, b=2, c=2, d=17)
        xq = blob2[:, 544:800].bitcast(DT.float32).rearrange(
            "p (i c) -> p i c", i=NIB, c=16)
        cwo1 = blob2[:, 800:802].bitcast(DT.float32)
        cone = blob2[:, 802:805].bitcast(DT.bfloat16)

        # start=True resets psum at larger-than-region granularity, so zero the
        # whole accumulator with one dummy all-zeros matmul and use start=False
        # (pure accumulate) for every real matmul.  One tile holds S (cols
        # 0:16 att, 16 c0) and C (col 17, T-g) so all writers are PE.
        P_ps = ps.tile([128, NIB, 18], DT.float32, tag="P", name="P")
        zmv = cpool.tile([128, 160], DT.bfloat16)
        nc.vector.memset(zmv[:], 0.0)
        nc.tensor.matmul(P_ps[:].rearrange("p a b -> p (a b)"), zmv[:, 0:128],
                         zmv[:, 0:NIB * 18], start=True, stop=False,
                         skip_group_check=True)

        last_tdma = None
        for pi, (t, row0, rows, ib0) in enumerate(PIECES):
            nib = rows // 128
            mt = mpool.tile([128, rows], DT.uint16, tag=f"mt{pi}", name=f"mt{pi}")
            last_tdma = nc.sync.dma_start(
                mt[:], mp_d[row0:row0 + rows, 128 * t:128 * (t + 1)],
                transpose=True)
            # bit-plane extracts: even keys = bit0, odd keys = bit1 (per byte)
            ev = mpool.tile([128, rows], DT.uint16, tag=f"ev{pi}", name=f"ev{pi}")
            nc.vector.tensor_scalar(ev[:], mt[:], 0x0101, 0, AL.bitwise_and,
                                    AL.bitwise_or)
            od = mpool.tile([128, rows], DT.uint16, tag=f"od{pi}", name=f"od{pi}")
            odx = nc.vector.tensor_scalar(od[:], mt[:], 1, 0x0101,
                                          AL.logical_shift_right, AL.bitwise_and)

            mr = mt[:].bitcast(DT.float8e4).rearrange("p (i two) -> p i two", two=2)
            evr = ev[:].bitcast(DT.float8e4).rearrange("p (i two) -> p i two", two=2)
            odr = od[:].bitcast(DT.float8e4).rearrange("p (i two) -> p i two", two=2)

            last = (pi == len(PIECES) - 1)
            for par in range(2):
                for e, pl in ((0, evr), (1, odr)):
                    stops = (last and par == 1 and e == 1)
                    for hl in range(2):
                        mv = w3[:, t, par, e, hl, :]
                        for k in range(nib):
                            nc.tensor.matmul(
                                P_ps[:, ib0 + k, 0:17],
                                pl[:, 128 * k:128 * (k + 1), par], mv,
                                start=False,
                                stop=(stops and hl == 1 and k == nib - 1),
                                skip_group_check=True)
                    # -g accumulation: C -= (1+e) * c0-plane contribution
                    for k in range(nib):
                        nc.tensor.matmul(
                            P_ps[:, ib0 + k, 17:18],
                            pl[:, 128 * k:128 * (k + 1), par],
                            cone[:, 1 + e:2 + e],
                            start=False, stop=False,
                            skip_group_check=True)
                # raw packed bytes vs ones: C += g + 4*c1
                for k in range(nib):
                    cmm = nc.tensor.matmul(
                        P_ps[:, ib0 + k, 17:18], mr[:, 128 * k:128 * (k + 1), par],
                        cone[:, 0:1],
                        start=False,
                        stop=(last and par == 1 and k == nib - 1),
                        skip_group_check=True)
                    # route deps through the chunk's DVE extract so the wait
                    # set collapses to a single DVE sem (covers mt + cone)
                    add_dep_helper(cmm.ins, odx.ins, reason="chunk ready")

        # ---- tail: scale + MLP, all in [128 queries, NIB, ch] layout ------
        # attended = S * w with w = 1/max(c0, 1): exact for c0 >= 1, and for
        # c0 == 0 S is exactly 0 so any finite w gives the reference 0.
        mcol = wpool.tile([128, NIB], DT.float32, tag="mcol")
        nc.vector.tensor_scalar_max(mcol[:], P_ps[:, :, 16], 1.0)
        wcol = wpool.tile([128, NIB], DT.float32, tag="wcol")
        nc.vector.reciprocal(wcol[:], mcol[:])

        # otA: [atts 16 | a+b 16 | a*b 16]; otB: [count 1 | a>b 16]
        otA = wpool.tile([128, NIB, 48], DT.float32, tag="otA")
        otB = wpool.tile([128, NIB, 17], DT.float32, tag="otB")
        wb = wcol[:].unsqueeze(2).broadcast_to([128, NIB, 16])
        nc.vector.tensor_tensor(otA[:, :, 0:16], P_ps[:, :, 0:16], wb, AL.mult)
        # MLP: a = x ch0:16, b = attended
        nc.vector.tensor_tensor(otA[:, :, 16:32], otA[:, :, 0:16], xq, AL.add)
        nc.vector.tensor_tensor(otA[:, :, 32:48], otA[:, :, 0:16], xq, AL.mult)
        # count: c1*W_o1 = (T - g) * (W_o1/4), C psum already holds T - g
        nc.vector.scalar_tensor_tensor(otB[:, :, 0], P_ps[:, :, 17], cwo1,
                                       wcol[:], AL.mult, AL.bypass)
        nc.vector.tensor_tensor(otB[:, :, 1:17], otA[:, :, 0:16], xq, AL.is_lt)

        # stores ride the Act queue set: a tiny Act fence absorbs the
        # xbar mode-switch wait (first regular DMA after the transposes), so
        # each store carries only its DVE data-dep sem (walrus allows one)
        fence = cpool.tile([1, 128], DT.uint16)
        fdma = nc.scalar.dma_start(fence[0:1, :], blob_d[0:1, :])
        add_dep_helper(fdma.ins, last_tdma.ins, reason="absorb xbar sem")
        sdma = nc.scalar.dma_start(outa_d[:], otA[:])
        add_dep_helper(sdma.ins, fdma.ins, sync=False, reason="queue order")
        # absorb storeB's DVE data dep into an Act engine op so the store's
        # single wait slot is free for its queue-predecessor sem
        babs = cpool.tile([1, 1], DT.float32)
        nc.scalar.copy(babs[:], otB[0:1, 0, 16:17])
        sdmb = nc.scalar.dma_start(outb_d[:], otB[:])
        add_dep_helper(sdmb.ins, sdma.ins, sync=False, reason="queue order")

    return nc


_cached = {}


def _prepare_in_maps(x, mask0, mask1, W_o0, W_o1):
    x = np.asarray(x, dtype=np.float32)
    m0 = np.asarray(mask0).view(np.uint8)
    m1 = np.asarray(mask1).view(np.uint8)
    W_o0 = np.asarray(W_o0, dtype=np.float32)
    W_o1 = np.asarray(W_o1, dtype=np.float32)

    # nibble pack: byte jj = m0[2jj] + 2*m0[2jj+1] + 4*(m1[2jj]+m1[2jj+1])
    packed = (m0[..., 0::2] + (m0[..., 1::2] << 1)
              + ((m1[..., 0::2] + m1[..., 1::2]) << 2))        # (B, S, S//2) u8

    # u = values through the head-0 output projection; hi/lo split, x512
    u = x[:, :, 0:16] @ W_o0.T                                 # (B, S, 16) f32
    u_hi = u.astype(ml_dtypes.bfloat16).astype(np.float32)
    u_lo = u - u_hi

    # key index per (partition, chunk, byte-lane, parity)
    p_i = np.arange(128)[:, None, None, None]
    t_i = np.arange(NCH)[None, :, None, None]
    par_i = np.arange(2)[None, None, :, None]
    e_i = np.arange(2)[None, None, None, :]
    J = 512 * t_i + 4 * p_i + 2 * par_i + e_i                  # [128,4,2,2]

    # cone cols: +512 (raw bytes -> g + 4*c1), -512 / -1024 (minus c0e / 2*c0o)
    cone = np.array([[512.0, -512.0, -1024.0]] * 128, dtype=ml_dtypes.bfloat16)
    cwo1 = np.full((128, 1), float(W_o1[0, 0]) / 4.0, dtype=np.float32)

    blobs = []
    for b in range(B):
        w3 = np.zeros((128, NCH, 2, 2, 2, 17), dtype=ml_dtypes.bfloat16)
        w3[..., 0, 0:16] = (512.0 * u_hi[b][J]).astype(ml_dtypes.bfloat16)
        w3[..., 1, 0:16] = (512.0 * u_lo[b][J]).astype(ml_dtypes.bfloat16)
        w3[..., 0, 16] = 512.0                                 # ones -> c0
        blobs.append(w3)

    in_maps = []
    for c in range(8):
        b, h = divmod(c, 2)
        sl = slice(QH * h, QH * (h + 1))
        xq = np.ascontiguousarray(
            x[b, sl, 0:16].reshape(NIB, 128, 16).transpose(1, 0, 2))
        blob = np.zeros((128, 1664), np.uint8)
        blob[:, 0:1088] = blobs[b].reshape(128, 544).view(np.uint8)
        blob[:, 1088:1600] = xq.reshape(128, 128).view(np.uint8)
        blob[:, 1600:1604] = cwo1.view(np.uint8)
        blob[:, 1604:1610] = cone.view(np.uint8)
        blob_t = np.ascontiguousarray(blob.view(np.uint16).T)  # [832, 128]
        in_maps.append({
            "mp": np.ascontiguousarray(packed[b, sl, :]).view(np.uint16),
            "blob": blob_t,
        })
    return in_maps


def kernel(x, mask0, mask1, W_o0, W_o1):
    if "nc" not in _cached:
        _cached["nc"] = _build_program()
    nc = _cached["nc"]
    in_maps = _prepare_in_maps(x, mask0, mask1, W_o0, W_o1)
    res = run_bass_kernel_spmd(nc, in_maps, list(range(8)))
    _cached["last_results"] = res
    out = np.array(np.asarray(x, dtype=np.float32), copy=True)
    for c in range(8):
        b, h = divmod(c, 2)
        sl = slice(QH * h, QH * (h + 1))
        ra = res.results[c]["outa"].transpose(1, 0, 2).reshape(QH, 48)
        rb = res.results[c]["outb"].transpose(1, 0, 2).reshape(QH, 17)
        out[b, sl, 16:32] = ra[:, 0:16]
        out[b, sl, 48:64] = ra[:, 16:32]
        out[b, sl, 64:80] = ra[:, 32:48]
        out[b, sl, 32] = rb[:, 0]
        out[b, sl, 80:96] = rb[:, 1:17]
    return out
